# revision 11
# baseline (speedup 1.0000x reference)
"""VQ-EMA codebook update kernel for Trainium2, 8 NeuronCores.

Strategy (data-parallel over tokens, per the standard VQ-EMA sync):
  - Each core gets N/8 = 4096 tokens; the [K=8192, C=384] dictionary is replicated.
  - P0: normalize dictionary rows (fp32, Newton-refined rsqrt), transpose via PE
        into ndT [C, K] stored as float32r (rounded) for fast PE matmuls.
  - P1: per 128-token tile: normalize tokens, transpose to nfT [C, 128] (f32r),
        sim = nfT.T @ ndT in fp32r (bf16-rate on PE, ~13-bit mantissa), PSUM->SBUF,
        row-max on DVE, one-hot = (sim >= rowmax) in bf16, spilled to DRAM.
  - P2: segment sums: for each K-tile, accumulate onehot.T @ [x|1] (bf16) over all
        token tiles in PSUM -> partial [K, C+1] (feature sums + counts).
  - P3: ReduceScatter(add) across the 8 cores -> each core owns a [1024, C+1]
        shard; EMA update + where(used) blend on-core; output its dict shard.
Host: shards inputs, concatenates the 8 output shards.
"""

import sys

sys.path.insert(0, "/opt/trn_rl_repo")

import functools

import numpy as np

N = 32768
C = 384
K = 8192
NCORES = 8
NSH = N // NCORES  # 4096 tokens per core
KSH = K // NCORES  # 1024 dict rows per core
TT = NSH // 128  # 32 token tiles per core
KT = K // 128  # 64 K tiles
CB = C // 128  # 3 contraction chunks
SIMW = 512  # sim matmul free width (PSUM bank, fp32)
NSIMW = K // SIMW  # 16 chunks
XW = C + 1  # 385: x plus ones column
MOM = 0.99


@functools.cache
def _build():
    import concourse.bacc as bacc
    import concourse.masks as masks
    import concourse.mybir as mybir
    import concourse.tile as tile

    f32 = mybir.dt.float32
    f32r = mybir.dt.float32r
    bf16 = mybir.dt.bfloat16

    nc = bacc.Bacc("TRN2", target_bir_lowering=False, debug=False, num_devices=NCORES)

    feat = nc.dram_tensor("feat", [NSH, C], f32, kind="ExternalInput").ap()
    dic = nc.dram_tensor("dic", [K, C], f32, kind="ExternalInput").ap()
    dsum = nc.dram_tensor("dsum", [KSH, C], f32, kind="ExternalInput").ap()
    dnum = nc.dram_tensor("dnum", [KSH // 128, 128, 1], f32, kind="ExternalInput").ap()
    dsh = nc.dram_tensor("dsh", [KSH, C], f32, kind="ExternalInput").ap()
    out_shard = nc.dram_tensor("out_shard", [KSH, C], f32, kind="ExternalOutput").ap()

    with tile.TileContext(nc) as tc:
        with (
            tc.tile_pool(name="constp", bufs=1) as constp,
            tc.tile_pool(name="mainp", bufs=1) as mainp,
            tc.tile_pool(name="dramp", bufs=1, space="DRAM") as dramp,
        ):
            ident = constp.tile([128, 128], f32, name="ident")
            masks.make_identity(nc, ident[:])

            # Persistent SBUF tensors
            ndT = [
                mainp.tile([128, K], f32r, name=f"ndT{c}", uniquify=False)
                for c in range(CB)
            ]
            # DRAM scratch
            onehot_dram = dramp.tile([NSH, K], bf16, name="onehot_dram")
            xext_dram = dramp.tile([NSH, XW], bf16, name="xext_dram")
            partial_dram = dramp.tile([K, XW], f32, name="partial_dram")
            ccout_dram = dramp.tile([KSH, XW], f32, name="ccout_dram")

            def rsqrt_refined(pool, ss, tag):
                """r ~= 1/sqrt(ss), fp32-accurate via 2 Newton steps on [128,1]."""
                rec = pool.tile([128, 1], f32, name=f"rec_{tag}", tag="rec")
                r = pool.tile([128, 1], f32, name=f"r_{tag}", tag="r")
                t = pool.tile([128, 1], f32, name=f"t_{tag}", tag="t")
                nc.vector.reciprocal(rec[:], ss[:])
                nc.scalar.sqrt(r[:], rec[:])
                for _ in range(2):
                    nc.vector.tensor_tensor(t[:], r[:], r[:], mybir.AluOpType.mult)
                    nc.vector.tensor_tensor(t[:], t[:], ss[:], mybir.AluOpType.mult)
                    # t = 1.5 - 0.5*t
                    nc.vector.tensor_scalar(
                        t[:], t[:], -0.5, 1.5, mybir.AluOpType.mult, mybir.AluOpType.add
                    )
                    nc.vector.tensor_tensor(r[:], r[:], t[:], mybir.AluOpType.mult)
                return r

            # ---------------- P0: dictionary normalize + transpose ----------------
            with (
                tc.tile_pool(name="p0sb", bufs=3) as p0sb,
                tc.tile_pool(name="p0sc", bufs=2) as p0sc,
                tc.tile_pool(name="p0ps", bufs=2, space="PSUM") as p0ps,
            ):
                for dt_i in range(KT):
                    d = p0sb.tile([128, C], f32, name="d", tag="d")
                    nc.sync.dma_start(d[:], dic[dt_i * 128 : (dt_i + 1) * 128, :])
                    sq = p0sc.tile([128, C], f32, name="sq", tag="sq")
                    ss = p0sc.tile([128, 1], f32, name="ss", tag="ss")
                    nc.scalar.activation(
                        sq[:], d[:], mybir.ActivationFunctionType.Square, accum_out=ss[:]
                    )
                    r = rsqrt_refined(p0sc, ss, "p0")
                    nd = p0sb.tile([128, C], f32, name="nd", tag="nd")
                    nc.scalar.activation(
                        nd[:], d[:], mybir.ActivationFunctionType.Copy, scale=r[:, 0:1]
                    )
                    for c in range(CB):
                        pst = p0ps.tile([128, 128], f32, name="pst", tag="pst")
                        nc.tensor.transpose(pst[:], nd[:, c * 128 : (c + 1) * 128], ident[:])
                        nc.vector.tensor_copy(
                            ndT[c][:, dt_i * 128 : (dt_i + 1) * 128], pst[:]
                        )

            # ---------------- P1: sim + one-hot per token tile ----------------
            with (
                tc.tile_pool(name="p1sb", bufs=3) as p1sb,
                tc.tile_pool(name="p1sc", bufs=2) as p1sc,
                tc.tile_pool(name="p1sim", bufs=2) as p1sim,
                tc.tile_pool(name="p1oh", bufs=1) as p1oh,
                tc.tile_pool(name="p1ps", bufs=4, space="PSUM") as p1ps,
                tc.tile_pool(name="p1pst", bufs=2, space="PSUM") as p1pst,
            ):
                for tt in range(TT):
                    x = p1sb.tile([128, C], f32, name="x", tag="x")
                    nc.sync.dma_start(x[:], feat[tt * 128 : (tt + 1) * 128, :])
                    sq = p1sc.tile([128, C], f32, name="sqx", tag="sqx")
                    ss = p1sc.tile([128, 1], f32, name="ssx", tag="ssx")
                    nc.scalar.activation(
                        sq[:], x[:], mybir.ActivationFunctionType.Square, accum_out=ss[:]
                    )
                    r = rsqrt_refined(p1sc, ss, "p1")
                    nf = p1sb.tile([128, C], f32, name="nf", tag="nf")
                    nc.scalar.activation(
                        nf[:], x[:], mybir.ActivationFunctionType.Copy, scale=r[:, 0:1]
                    )
                    # raw x (bf16) + ones column staged and spilled to DRAM
                    xe = p1sb.tile([128, XW], bf16, name="xe", tag="xe")
                    nc.vector.memset(xe[:, C:XW], 1.0)
                    nc.vector.tensor_copy(xe[:, 0:C], x[:])
                    nc.sync.dma_start(xext_dram[tt * 128 : (tt + 1) * 128, :], xe[:])

                    nfT = []
                    for c in range(CB):
                        pst = p1pst.tile([128, 128], f32, name="pstx", tag="pstx")
                        nc.tensor.transpose(pst[:], nf[:, c * 128 : (c + 1) * 128], ident[:])
                        nfc = p1sb.tile([128, 128], f32r, name="nfc", tag=f"nfc{c}")
                        nc.scalar.copy(nfc[:], pst[:])
                        nfT.append(nfc)

                    simbuf = p1sim.tile([128, K], f32, name="simbuf", tag="simbuf")
                    for kc in range(NSIMW):
                        ps = p1ps.tile([128, SIMW], f32, name="ps_sim", tag="ps_sim")
                        for c in range(CB):
                            nc.tensor.matmul(
                                ps[:],
                                nfT[c][:],
                                ndT[c][:, kc * SIMW : (kc + 1) * SIMW],
                                start=(c == 0),
                                stop=(c == CB - 1),
                            )
                        nc.scalar.copy(simbuf[:, kc * SIMW : (kc + 1) * SIMW], ps[:])

                    rowmax = p1sc.tile([128, 1], f32, name="rowmax", tag="rowmax")
                    nc.vector.tensor_reduce(
                        rowmax[:], simbuf[:], mybir.AxisListType.X, mybir.AluOpType.max
                    )
                    onehot = p1oh.tile([128, K], bf16, name="onehot", tag="onehot")
                    nc.vector.tensor_scalar(
                        onehot[:], simbuf[:], rowmax[:, 0:1], None, mybir.AluOpType.is_ge
                    )
                    nc.sync.dma_start(
                        onehot_dram[tt * 128 : (tt + 1) * 128, :], onehot[:]
                    )

            # ---------------- P2: segment sums via one-hot matmuls ----------------
            with (
                tc.tile_pool(name="p2oh", bufs=6) as p2oh,
                tc.tile_pool(name="p2xe", bufs=4) as p2xe,
                tc.tile_pool(name="p2st", bufs=2) as p2st,
                tc.tile_pool(name="p2ps", bufs=8, space="PSUM") as p2ps,
            ):
                for g in range(8):
                    segs = [
                        p2ps.tile([128, XW], f32, name=f"ps_seg{b}", tag="ps_seg")
                        for b in range(8)
                    ]
                    for tt in range(TT):
                        oh = p2oh.tile([128, 1024], bf16, name="oh", tag="oh")
                        nc.sync.dma_start(
                            oh[:],
                            onehot_dram[
                                tt * 128 : (tt + 1) * 128, g * 1024 : (g + 1) * 1024
                            ],
                        )
                        xe2 = p2xe.tile([128, XW], bf16, name="xe2", tag="xe2")
                        nc.sync.dma_start(
                            xe2[:], xext_dram[tt * 128 : (tt + 1) * 128, :]
                        )
                        for b in range(8):
                            nc.tensor.matmul(
                                segs[b][:],
                                oh[:, b * 128 : (b + 1) * 128],
                                xe2[:],
                                start=(tt == 0),
                                stop=(tt == TT - 1),
                            )
                    for b in range(8):
                        stg = p2st.tile([128, XW], f32, name="stg", tag="stg")
                        nc.scalar.copy(stg[:], segs[b][:])
                        kt = g * 8 + b
                        nc.sync.dma_start(
                            partial_dram[kt * 128 : (kt + 1) * 128, :], stg[:]
                        )

            # ---------------- P3: reduce-scatter + EMA ----------------
            if globals().get("SKIP_COLLECTIVE", False):
                nc.sync.dma_start(ccout_dram[:], partial_dram[0:KSH, :])
            else:
                nc.gpsimd.collective_compute(
                    "ReduceScatter",
                    mybir.AluOpType.add,
                    replica_groups=[list(range(NCORES))],
                    ins=[partial_dram.opt()],
                    outs=[ccout_dram.opt()],
                )
            with tc.tile_pool(name="p3sb", bufs=2) as p3sb:
                for st in range(KSH // 128):
                    red = p3sb.tile([128, XW], f32, name="red", tag="red")
                    nc.sync.dma_start(red[:], ccout_dram[st * 128 : (st + 1) * 128, :])
                    dsum_t = p3sb.tile([128, C], f32, name="dsum_t", tag="dsum_t")
                    nc.sync.dma_start(dsum_t[:], dsum[st * 128 : (st + 1) * 128, :])
                    dnum_t = p3sb.tile([128, 1], f32, name="dnum_t", tag="dnum_t")
                    nc.sync.dma_start(dnum_t[:], dnum[st, :, :])
                    dsh_t = p3sb.tile([128, C], f32, name="dsh_t", tag="dsh_t")
                    nc.sync.dma_start(dsh_t[:], dsh[st * 128 : (st + 1) * 128, :])

                    cnt = red[:, C : C + 1]
                    maskb = p3sb.tile([128, 1], f32, name="maskb", tag="maskb")
                    nc.vector.tensor_scalar(
                        maskb[:], cnt, 0.0, None, mybir.AluOpType.is_gt
                    )
                    mask001 = p3sb.tile([128, 1], f32, name="mask001", tag="mask001")
                    nc.vector.tensor_scalar(
                        mask001[:],
                        cnt,
                        0.0,
                        1.0 - MOM,
                        mybir.AluOpType.is_gt,
                        mybir.AluOpType.mult,
                    )

                    # new_sum = dsum + mask001 * (ema - dsum)
                    tmp = p3sb.tile([128, C], f32, name="tmp", tag="tmp")
                    nc.vector.tensor_tensor(
                        tmp[:], red[:, 0:C], dsum_t[:], mybir.AluOpType.subtract
                    )
                    nc.vector.tensor_scalar(
                        tmp[:], tmp[:], mask001[:, 0:1], None, mybir.AluOpType.mult
                    )
                    nsum = p3sb.tile([128, C], f32, name="nsum", tag="nsum")
                    nc.vector.tensor_tensor(
                        nsum[:], tmp[:], dsum_t[:], mybir.AluOpType.add
                    )

                    # new_num = dnum + mask001 * (cnt - dnum)
                    n0 = p3sb.tile([128, 1], f32, name="n0", tag="n0")
                    nc.vector.tensor_tensor(
                        n0[:], cnt, dnum_t[:], mybir.AluOpType.subtract
                    )
                    nc.vector.tensor_tensor(
                        n0[:], n0[:], mask001[:], mybir.AluOpType.mult
                    )
                    nnum = p3sb.tile([128, 1], f32, name="nnum", tag="nnum")
                    nc.vector.tensor_tensor(
                        nnum[:], n0[:], dnum_t[:], mybir.AluOpType.add
                    )
                    rec = p3sb.tile([128, 1], f32, name="recq", tag="recq")
                    nc.vector.reciprocal(rec[:], nnum[:])

                    # q = new_sum / new_num ; out = dsh + maskb * (q - dsh)
                    q = p3sb.tile([128, C], f32, name="q", tag="q")
                    nc.vector.tensor_scalar(
                        q[:], nsum[:], rec[:, 0:1], None, mybir.AluOpType.mult
                    )
                    nc.vector.tensor_tensor(
                        q[:], q[:], dsh_t[:], mybir.AluOpType.subtract
                    )
                    nc.vector.tensor_scalar(
                        q[:], q[:], maskb[:, 0:1], None, mybir.AluOpType.mult
                    )
                    outt = p3sb.tile([128, C], f32, name="outt", tag="outt")
                    nc.vector.tensor_tensor(
                        outt[:], q[:], dsh_t[:], mybir.AluOpType.add
                    )
                    nc.sync.dma_start(
                        out_shard[st * 128 : (st + 1) * 128, :], outt[:]
                    )

    nc.compile()
    return nc


def kernel(feature, dictionary, dictionary_sum, dictionary_num):
    from concourse import bass_utils

    feature = np.ascontiguousarray(feature, dtype=np.float32)
    dictionary = np.ascontiguousarray(dictionary, dtype=np.float32)
    dictionary_sum = np.ascontiguousarray(dictionary_sum, dtype=np.float32)
    dictionary_num = np.ascontiguousarray(dictionary_num, dtype=np.float32)

    nc = _build()
    in_maps = []
    for i in range(NCORES):
        in_maps.append(
            {
                "feat": feature[i * NSH : (i + 1) * NSH],
                "dic": dictionary,
                "dsum": dictionary_sum[i * KSH : (i + 1) * KSH],
                "dnum": dictionary_num[i * KSH : (i + 1) * KSH].reshape(
                    KSH // 128, 128, 1
                ),
                "dsh": dictionary[i * KSH : (i + 1) * KSH],
            }
        )
    res = bass_utils.run_bass_kernel_spmd(nc, in_maps, core_ids=list(range(NCORES)))
    out = np.concatenate(
        [res.results[i]["out_shard"] for i in range(NCORES)], axis=0
    )
    return out.astype(np.float32)


# revision 12
# speedup vs baseline: 1.9535x; 1.9535x over previous
"""VQ-EMA codebook update kernel for Trainium2, 8 NeuronCores.

Strategy (data-parallel over tokens, per the standard VQ-EMA sync):
  - Each core gets N/8 = 4096 tokens; the [K=8192, C=384] dictionary is replicated.
  - P0: normalize dictionary rows (fp32, Newton-refined rsqrt), transpose via PE
        into ndT [C, K] stored as float32r (rounded) for fast PE matmuls.
  - P1: per 128-token tile: normalize tokens, transpose to nfT [C, 128] (f32r),
        sim = nfT.T @ ndT in fp32r (bf16-rate on PE, ~13-bit mantissa), PSUM->SBUF,
        row-max on DVE, one-hot = (sim >= rowmax) in bf16, spilled to DRAM.
  - P2: segment sums: for each K-tile, accumulate onehot.T @ [x|1] (bf16) over all
        token tiles in PSUM -> partial [K, C+1] (feature sums + counts).
  - P3: ReduceScatter(add) across the 8 cores -> each core owns a [1024, C+1]
        shard; EMA update + where(used) blend on-core; output its dict shard.
Host: shards inputs, concatenates the 8 output shards.
"""

import sys

sys.path.insert(0, "/opt/trn_rl_repo")

import functools

import numpy as np

N = 32768
C = 384
K = 8192
NCORES = 8
NSH = N // NCORES  # 4096 tokens per core
KSH = K // NCORES  # 1024 dict rows per core
TT = NSH // 128  # 32 token tiles per core
KT = K // 128  # 64 K tiles
CB = C // 128  # 3 contraction chunks
SIMW = 512  # sim matmul free width (PSUM bank, fp32)
NSIMW = K // SIMW  # 16 chunks
XW = C + 1  # 385: x plus ones column
MOM = 0.99


@functools.cache
def _build():
    import concourse.bacc as bacc
    import concourse.masks as masks
    import concourse.mybir as mybir
    import concourse.tile as tile

    f32 = mybir.dt.float32
    f32r = mybir.dt.float32r
    bf16 = mybir.dt.bfloat16

    nc = bacc.Bacc("TRN2", target_bir_lowering=False, debug=False, num_devices=NCORES)

    feat = nc.dram_tensor("feat", [NSH, C], f32, kind="ExternalInput").ap()
    dic = nc.dram_tensor("dic", [K, C], f32, kind="ExternalInput").ap()
    dsum = nc.dram_tensor("dsum", [KSH, C], f32, kind="ExternalInput").ap()
    dnum = nc.dram_tensor("dnum", [KSH // 128, 128, 1], f32, kind="ExternalInput").ap()
    dsh = nc.dram_tensor("dsh", [KSH, C], f32, kind="ExternalInput").ap()
    out_shard = nc.dram_tensor("out_shard", [KSH, C], f32, kind="ExternalOutput").ap()

    with tile.TileContext(nc) as tc:
        with (
            tc.tile_pool(name="constp", bufs=1) as constp,
            tc.tile_pool(name="mainp", bufs=1) as mainp,
            tc.tile_pool(name="dramp", bufs=1, space="DRAM") as dramp,
        ):
            ident = constp.tile([128, 128], f32, name="ident")
            masks.make_identity(nc, ident[:])

            # Persistent SBUF tensors
            ndT = [
                mainp.tile([128, K], f32r, name=f"ndT{c}", uniquify=False)
                for c in range(CB)
            ]
            # DRAM scratch
            onehot_dram = dramp.tile([NSH, K], bf16, name="onehot_dram")
            xext_dram = dramp.tile([NSH, XW], bf16, name="xext_dram")
            partial_dram = dramp.tile([K, XW], f32, name="partial_dram")
            ccout_dram = dramp.tile([KSH, XW], f32, name="ccout_dram")

            def rsqrt_refined(pool, ss, tag):
                """r ~= 1/sqrt(ss), fp32-accurate via 2 Newton steps on [128,1]."""
                rec = pool.tile([128, 1], f32, name=f"rec_{tag}", tag="rec")
                r = pool.tile([128, 1], f32, name=f"r_{tag}", tag="r")
                t = pool.tile([128, 1], f32, name=f"t_{tag}", tag="t")
                nc.vector.reciprocal(rec[:], ss[:])
                nc.scalar.sqrt(r[:], rec[:])
                for _ in range(2):
                    nc.vector.tensor_tensor(t[:], r[:], r[:], mybir.AluOpType.mult)
                    nc.vector.tensor_tensor(t[:], t[:], ss[:], mybir.AluOpType.mult)
                    # t = 1.5 - 0.5*t
                    nc.vector.tensor_scalar(
                        t[:], t[:], -0.5, 1.5, mybir.AluOpType.mult, mybir.AluOpType.add
                    )
                    nc.vector.tensor_tensor(r[:], r[:], t[:], mybir.AluOpType.mult)
                return r

            # ---------------- P0: dictionary normalize + transpose ----------------
            with (
                tc.tile_pool(name="p0sb", bufs=3) as p0sb,
                tc.tile_pool(name="p0sc", bufs=2) as p0sc,
                tc.tile_pool(name="p0ps", bufs=2, space="PSUM") as p0ps,
            ):
                for dt_i in range(KT):
                    d = p0sb.tile([128, C], f32, name="d", tag="d")
                    nc.sync.dma_start(d[:], dic[dt_i * 128 : (dt_i + 1) * 128, :])
                    sq = p0sc.tile([128, C], f32, name="sq", tag="sq")
                    ss = p0sc.tile([128, 1], f32, name="ss", tag="ss")
                    nc.scalar.activation(
                        sq[:], d[:], mybir.ActivationFunctionType.Square, accum_out=ss[:]
                    )
                    r = rsqrt_refined(p0sc, ss, "p0")
                    nd = p0sb.tile([128, C], f32, name="nd", tag="nd")
                    nc.scalar.activation(
                        nd[:], d[:], mybir.ActivationFunctionType.Copy, scale=r[:, 0:1]
                    )
                    for c in range(CB):
                        pst = p0ps.tile([128, 128], f32, name="pst", tag="pst")
                        nc.tensor.transpose(pst[:], nd[:, c * 128 : (c + 1) * 128], ident[:])
                        nc.vector.tensor_copy(
                            ndT[c][:, dt_i * 128 : (dt_i + 1) * 128], pst[:]
                        )

            # ---------------- P1: sim + one-hot per token tile ----------------
            with (
                tc.tile_pool(name="p1sb", bufs=3) as p1sb,
                tc.tile_pool(name="p1sc", bufs=2) as p1sc,
                tc.tile_pool(name="p1sim", bufs=2) as p1sim,
                tc.tile_pool(name="p1oh", bufs=1) as p1oh,
                tc.tile_pool(name="p1ps", bufs=4, space="PSUM") as p1ps,
                tc.tile_pool(name="p1pst", bufs=2, space="PSUM") as p1pst,
            ):
                for tt in range(TT):
                    x = p1sb.tile([128, C], f32, name="x", tag="x")
                    nc.sync.dma_start(x[:], feat[tt * 128 : (tt + 1) * 128, :])
                    sq = p1sc.tile([128, C], f32, name="sqx", tag="sqx")
                    ss = p1sc.tile([128, 1], f32, name="ssx", tag="ssx")
                    nc.scalar.activation(
                        sq[:], x[:], mybir.ActivationFunctionType.Square, accum_out=ss[:]
                    )
                    r = rsqrt_refined(p1sc, ss, "p1")
                    nf = p1sb.tile([128, C], f32, name="nf", tag="nf")
                    nc.scalar.activation(
                        nf[:], x[:], mybir.ActivationFunctionType.Copy, scale=r[:, 0:1]
                    )
                    # raw x (bf16) + ones column staged and spilled to DRAM
                    xe = p1sb.tile([128, XW], bf16, name="xe", tag="xe")
                    nc.vector.memset(xe[:, C:XW], 1.0)
                    nc.vector.tensor_copy(xe[:, 0:C], x[:])
                    nc.sync.dma_start(xext_dram[tt * 128 : (tt + 1) * 128, :], xe[:])

                    nfT = []
                    for c in range(CB):
                        pst = p1pst.tile([128, 128], f32, name="pstx", tag="pstx")
                        nc.tensor.transpose(pst[:], nf[:, c * 128 : (c + 1) * 128], ident[:])
                        nfc = p1sb.tile([128, 128], f32r, name="nfc", tag=f"nfc{c}")
                        nc.scalar.copy(nfc[:], pst[:])
                        nfT.append(nfc)

                    simbuf = p1sim.tile([128, K], f32, name="simbuf", tag="simbuf")
                    for kc in range(NSIMW):
                        ps = p1ps.tile([128, SIMW], f32, name="ps_sim", tag="ps_sim")
                        for c in range(CB):
                            nc.tensor.matmul(
                                ps[:],
                                nfT[c][:],
                                ndT[c][:, kc * SIMW : (kc + 1) * SIMW],
                                start=(c == 0),
                                stop=(c == CB - 1),
                            )
                        nc.scalar.copy(simbuf[:, kc * SIMW : (kc + 1) * SIMW], ps[:])

                    rowmax = p1sc.tile([128, 1], f32, name="rowmax", tag="rowmax")
                    nc.vector.tensor_reduce(
                        rowmax[:], simbuf[:], mybir.AxisListType.X, mybir.AluOpType.max
                    )
                    onehot = p1oh.tile([128, K], bf16, name="onehot", tag="onehot")
                    nc.vector.tensor_scalar(
                        onehot[:], simbuf[:], rowmax[:, 0:1], None, mybir.AluOpType.is_ge
                    )
                    nc.sync.dma_start(
                        onehot_dram[tt * 128 : (tt + 1) * 128, :], onehot[:]
                    )

            # ---------------- P2: segment sums via one-hot matmuls ----------------
            with (
                tc.tile_pool(name="p2oh", bufs=6) as p2oh,
                tc.tile_pool(name="p2xe", bufs=4) as p2xe,
                tc.tile_pool(name="p3sb", bufs=2) as p3sb,
                tc.tile_pool(name="p2st", bufs=2) as p2st,
                tc.tile_pool(name="p2ps", bufs=8, space="PSUM") as p2ps,
            ):
                for g in range(8):
                    segs = [
                        p2ps.tile([128, XW], f32, name=f"ps_seg{b}", tag="ps_seg")
                        for b in range(8)
                    ]
                    for tt in range(TT):
                        oh = p2oh.tile([128, 1024], bf16, name="oh", tag="oh")
                        nc.sync.dma_start(
                            oh[:],
                            onehot_dram[
                                tt * 128 : (tt + 1) * 128, g * 1024 : (g + 1) * 1024
                            ],
                        )
                        xe2 = p2xe.tile([128, XW], bf16, name="xe2", tag="xe2")
                        nc.sync.dma_start(
                            xe2[:], xext_dram[tt * 128 : (tt + 1) * 128, :]
                        )
                        for b in range(8):
                            nc.tensor.matmul(
                                segs[b][:],
                                oh[:, b * 128 : (b + 1) * 128],
                                xe2[:],
                                start=(tt == 0),
                                stop=(tt == TT - 1),
                            )
                    for b in range(8):
                        stg = p2st.tile([128, XW], f32, name="stg", tag="stg")
                        nc.scalar.copy(stg[:], segs[b][:])
                        kt = g * 8 + b
                        nc.sync.dma_start(
                            partial_dram[kt * 128 : (kt + 1) * 128, :], stg[:]
                        )
                    # per-group ReduceScatter: overlaps later groups' matmuls on TOPSP.
                    # rank i receives rows [g*1024 + i*128, +128) -> ccout[g*128:(g+1)*128]
                    if globals().get("SKIP_COLLECTIVE", False):
                        nc.sync.dma_start(
                            ccout_dram[g * 128 : (g + 1) * 128, :],
                            partial_dram[g * 1024 : g * 1024 + 128, :],
                        )
                    else:
                        nc.gpsimd.collective_compute(
                            "ReduceScatter",
                            mybir.AluOpType.add,
                            replica_groups=[list(range(NCORES))],
                            ins=[partial_dram[g * 1024 : (g + 1) * 1024, :].opt()],
                            outs=[ccout_dram[g * 128 : (g + 1) * 128, :].opt()],
                        )
                    st = g
                    red = p3sb.tile([128, XW], f32, name="red", tag="red")
                    nc.sync.dma_start(red[:], ccout_dram[st * 128 : (st + 1) * 128, :])
                    dsum_t = p3sb.tile([128, C], f32, name="dsum_t", tag="dsum_t")
                    nc.sync.dma_start(dsum_t[:], dsum[st * 128 : (st + 1) * 128, :])
                    dnum_t = p3sb.tile([128, 1], f32, name="dnum_t", tag="dnum_t")
                    nc.sync.dma_start(dnum_t[:], dnum[st, :, :])
                    dsh_t = p3sb.tile([128, C], f32, name="dsh_t", tag="dsh_t")
                    nc.sync.dma_start(dsh_t[:], dsh[st * 128 : (st + 1) * 128, :])

                    cnt = red[:, C : C + 1]
                    maskb = p3sb.tile([128, 1], f32, name="maskb", tag="maskb")
                    nc.vector.tensor_scalar(
                        maskb[:], cnt, 0.0, None, mybir.AluOpType.is_gt
                    )
                    mask001 = p3sb.tile([128, 1], f32, name="mask001", tag="mask001")
                    nc.vector.tensor_scalar(
                        mask001[:], cnt, 0.0, 1.0 - MOM,
                        mybir.AluOpType.is_gt, mybir.AluOpType.mult,
                    )
                    tmp = p3sb.tile([128, C], f32, name="tmp", tag="tmp")
                    nc.vector.tensor_tensor(
                        tmp[:], red[:, 0:C], dsum_t[:], mybir.AluOpType.subtract
                    )
                    nc.vector.tensor_scalar(
                        tmp[:], tmp[:], mask001[:, 0:1], None, mybir.AluOpType.mult
                    )
                    nsum = p3sb.tile([128, C], f32, name="nsum", tag="nsum")
                    nc.vector.tensor_tensor(
                        nsum[:], tmp[:], dsum_t[:], mybir.AluOpType.add
                    )
                    n0 = p3sb.tile([128, 1], f32, name="n0", tag="n0")
                    nc.vector.tensor_tensor(
                        n0[:], cnt, dnum_t[:], mybir.AluOpType.subtract
                    )
                    nc.vector.tensor_tensor(
                        n0[:], n0[:], mask001[:], mybir.AluOpType.mult
                    )
                    nnum = p3sb.tile([128, 1], f32, name="nnum", tag="nnum")
                    nc.vector.tensor_tensor(
                        nnum[:], n0[:], dnum_t[:], mybir.AluOpType.add
                    )
                    rec = p3sb.tile([128, 1], f32, name="recq", tag="recq")
                    nc.vector.reciprocal(rec[:], nnum[:])
                    q = p3sb.tile([128, C], f32, name="q", tag="q")
                    nc.vector.tensor_scalar(
                        q[:], nsum[:], rec[:, 0:1], None, mybir.AluOpType.mult
                    )
                    nc.vector.tensor_tensor(
                        q[:], q[:], dsh_t[:], mybir.AluOpType.subtract
                    )
                    nc.vector.tensor_scalar(
                        q[:], q[:], maskb[:, 0:1], None, mybir.AluOpType.mult
                    )
                    outt = p3sb.tile([128, C], f32, name="outt", tag="outt")
                    nc.vector.tensor_tensor(
                        outt[:], q[:], dsh_t[:], mybir.AluOpType.add
                    )
                    nc.sync.dma_start(
                        out_shard[st * 128 : (st + 1) * 128, :], outt[:]
                    )

    nc.compile()
    return nc


def _shard_rows(i):
    """Global dictionary rows owned by core i: the i-th 128-block of each group."""
    return [(g * KSH + i * 128, g * KSH + i * 128 + 128) for g in range(KSH // 128)]


def shard_inputs(feature, dictionary, dictionary_sum, dictionary_num):
    in_maps = []
    for i in range(NCORES):
        rows = _shard_rows(i)
        dsum_i = np.concatenate([dictionary_sum[a:b] for a, b in rows], axis=0)
        dsh_i = np.concatenate([dictionary[a:b] for a, b in rows], axis=0)
        dnum_i = np.concatenate([dictionary_num[a:b] for a, b in rows], axis=0)
        in_maps.append(
            {
                "feat": np.ascontiguousarray(feature[i * NSH : (i + 1) * NSH]),
                "dic": dictionary,
                "dsum": np.ascontiguousarray(dsum_i),
                "dnum": np.ascontiguousarray(dnum_i).reshape(KSH // 128, 128, 1),
                "dsh": np.ascontiguousarray(dsh_i),
            }
        )
    return in_maps


def unshard_output(results):
    out = np.empty((K, C), np.float32)
    for i in range(NCORES):
        rows = _shard_rows(i)
        for g, (a, b) in enumerate(rows):
            out[a:b] = results[i]["out_shard"][g * 128 : (g + 1) * 128]
    return out


def kernel(feature, dictionary, dictionary_sum, dictionary_num):
    from concourse import bass_utils

    feature = np.ascontiguousarray(feature, dtype=np.float32)
    dictionary = np.ascontiguousarray(dictionary, dtype=np.float32)
    dictionary_sum = np.ascontiguousarray(dictionary_sum, dtype=np.float32)
    dictionary_num = np.ascontiguousarray(dictionary_num, dtype=np.float32)

    nc = _build()
    in_maps = shard_inputs(feature, dictionary, dictionary_sum, dictionary_num)
    res = bass_utils.run_bass_kernel_spmd(nc, in_maps, core_ids=list(range(NCORES)))
    return unshard_output(res.results).astype(np.float32)
